# revision 7
# baseline (speedup 1.0000x reference)
"""Causal-mask multi-head attention (B=2, S=2048, D=1024, H=16) on 8 trn2
NeuronCores.

Sharding: core c = 4*b + g handles batch b and head-group g (4 heads).
Each core computes q/k/v projections for its head group (column-sliced
weights), block-causal attention over its batch, and the partial output
projection with its row-slice of wo.  The host sums the 4 per-batch
partials -- no device collectives.

Kernel structure (v2):
  - All matmuls bf16 (1 cycle/row on PE regardless of tile width).
  - ST (logits^T) per key-block PAIR into one [128, 2, 512] psum tile so a
    single exp activation covers 1024 columns.
  - Causal mask on diagonal blocks via a bf16 ident@maskb accumulation.
  - AV in "orientation B": stationary = P^T chunk [128k, 128q], moving =
    v4 [128k, 65] -> psum [128q, 64hd+Z].  Output partitions fully used and
    the softmax denominator Z lands per-partition, so normalization is a
    cheap per-partition tensor op (no cross-partition broadcast).
  - Normalized attention rows are PE-transposed back to [hd, q] for the
    output projection.
"""
import numpy as np
import ml_dtypes

_bf16 = ml_dtypes.bfloat16

import concourse.bass as bass
import concourse.tile as tile
import concourse.mybir as mybir
from concourse import bacc
from concourse.bass_utils import run_bass_kernel_spmd

B, S, D = 2, 2048, 1024
H, DH = 16, 64
HG = 4                 # heads per core
DG = HG * DH           # 256 projection cols per core
P = 128
QW = 512               # query window
NW = S // QW           # 4 windows
NKB = S // P           # 16 key blocks
NC = D // P            # 8 contraction chunks of d_model
NRC = S // P           # 16 row chunks
F32 = mybir.dt.float32
F32R = mybir.dt.float32r
BF16 = mybir.dt.bfloat16

_cached_nc = None


def _build_nc():
    nc = bacc.Bacc("TRN2", target_bir_lowering=False, debug=False, num_devices=8)

    xqT = nc.dram_tensor("xqT", [D, S], BF16, kind="ExternalInput").ap()
    xkT = nc.dram_tensor("xkT", [D, S], BF16, kind="ExternalInput").ap()
    xvT = nc.dram_tensor("xvT", [D, S], BF16, kind="ExternalInput").ap()
    wq = nc.dram_tensor("wq", [D, DG], BF16, kind="ExternalInput").ap()
    wk = nc.dram_tensor("wk", [D, DG], BF16, kind="ExternalInput").ap()
    # wv extended with a zero column per head (65th col); ones row added via
    # bias matmul below -> v4 holds [v, 1] per head.
    wve = nc.dram_tensor("wve", [D, HG * (DH + 1)], BF16, kind="ExternalInput").ap()
    bq = nc.dram_tensor("bq", [DG], F32, kind="ExternalInput").ap()
    bk = nc.dram_tensor("bk", [DG], F32, kind="ExternalInput").ap()
    bvz = nc.dram_tensor("bvz", [1, HG * (DH + 1)], F32R, kind="ExternalInput").ap()
    wo = nc.dram_tensor("wo", [DG, D], BF16, kind="ExternalInput").ap()
    bo = nc.dram_tensor("bo", [D], F32, kind="ExternalInput").ap()
    maskb = nc.dram_tensor("maskb", [P, P], BF16, kind="ExternalInput").ap()
    identb = nc.dram_tensor("identb", [P, P], BF16, kind="ExternalInput").ap()
    onesd = nc.dram_tensor("onesd", [1, P], F32R, kind="ExternalInput").ap()
    out = nc.dram_tensor("out", [S, D], BF16, kind="ExternalOutput").ap()

    VCOL = HG * (DH + 1)   # 260

    from contextlib import ExitStack
    with tile.TileContext(nc) as tc, ExitStack() as ctx:
        consts = ctx.enter_context(tc.tile_pool(name="consts", bufs=1))
        slabs = ctx.enter_context(tc.tile_pool(name="slabs", bufs=9))
        persist = ctx.enter_context(tc.tile_pool(name="persist", bufs=1))
        ptp = ctx.enter_context(tc.tile_pool(name="ptp", bufs=1))

        # ---- constants / weights in SBUF (issue order = need order) ----
        wk_sb = consts.tile([P, NC, DG], BF16)
        nc.sync.dma_start(wk_sb[:], wk.rearrange("(c p) m -> p c m", p=P))
        wq_sb = consts.tile([P, NC, DG], BF16)
        nc.sync.dma_start(wq_sb[:], wq.rearrange("(c p) m -> p c m", p=P))
        maskb_sb = consts.tile([P, P], BF16)
        nc.sync.dma_start(maskb_sb[:], maskb)
        ident_sb = consts.tile([P, P], BF16)
        nc.sync.dma_start(ident_sb[:], identb)
        bk_sb = consts.tile([P, 2], F32)
        nc.sync.dma_start(bk_sb[:], bk.rearrange("(t p) -> p t", p=P))
        bq_sb = consts.tile([P, 2], F32)
        nc.sync.dma_start(bq_sb[:], bq.rearrange("(t p) -> p t", p=P))
        wv_sb = consts.tile([P, NC, VCOL], BF16)
        nc.sync.dma_start(wv_sb[:], wve.rearrange("(c p) m -> p c m", p=P))
        bvz_sb = consts.tile([1, VCOL], F32R)
        nc.sync.dma_start(bvz_sb[:], bvz)
        ones1 = consts.tile([1, P], F32R)
        nc.sync.dma_start(ones1[:], onesd)
        wo_sb = consts.tile([P, 2, D], BF16)
        nc.sync.dma_start(wo_sb[:], wo.rearrange("(t p) m -> p t m", p=P))
        bo_bc = consts.tile([P, D], F32)
        nc.sync.dma_start(bo_bc[:], bass.AP(
            tensor=bo.tensor, offset=0, ap=[[0, P], [1, D]]))

        # persistent activation storage
        qT = [persist.tile([P, S], BF16, tag=f"qT{t}", name=f"qT{t}") for t in range(2)]
        kT = [persist.tile([P, S], BF16, tag=f"kT{t}", name=f"kT{t}") for t in range(2)]
        # v4[p, rc, h*65+j]: v rows (key pos within block rc) x (4 heads x
        # [64 v dims, 1 ones-col])
        v4 = persist.tile([P, NRC, VCOL], BF16, tag="v4")
        aoT = persist.tile([P, 2, S], BF16, tag="aoT")
        # P^T tiles: pt[h][j] covers key-block pair (2j, 2j+1) for head h,
        # all 512 query columns of the current window.
        pt = [[ptp.tile([P, 2, QW], BF16, tag=f"pt{h}_{j}", name=f"pt{h}_{j}")
               for j in range(NKB // 2)] for h in range(HG)]

        # ---------------- phase 1: projections ----------------
        with tc.tile_pool(name="gp", bufs=4, space="PSUM") as gp:
            # k then q: out rows = head dims (2 tiles of 128), cols = seq
            for name, src, w_sb, b_sb, dst in (
                ("k", xkT, wk_sb, bk_sb, kT),
                ("q", xqT, wq_sb, bq_sb, qT),
            ):
                xsl = []
                for c in range(NC):
                    slab = slabs.tile([P, S], BF16, tag="slab")
                    nc.sync.dma_start(slab[:, 0:S // 2],
                                      src[c * P:(c + 1) * P, 0:S // 2])
                    nc.sync.dma_start(slab[:, S // 2:S],
                                      src[c * P:(c + 1) * P, S // 2:S])
                    xsl.append(slab)
                for t in range(2):
                    for w in range(NW):
                        ps = gp.tile([P, QW], F32, tag="gp")
                        for c in range(NC):
                            nc.tensor.matmul(
                                ps[:],
                                w_sb[:, c, t * P:(t + 1) * P],
                                xsl[c][:, w * QW:(w + 1) * QW],
                                start=(c == 0), stop=(c == NC - 1),
                            )
                        if w % 2 == 0:
                            nc.vector.tensor_scalar_add(
                                dst[t][:, w * QW:(w + 1) * QW],
                                ps[:], b_sb[:, t:t + 1])
                        else:
                            nc.scalar.activation(
                                dst[t][:, w * QW:(w + 1) * QW],
                                ps[:],
                                mybir.ActivationFunctionType.Identity,
                                bias=b_sb[:, t:t + 1])
            # v: rows = seq (16 chunks), cols = 4 heads x 65
            vsl = []
            for c in range(NC):
                slab = slabs.tile([P, S], BF16, tag="slab")
                nc.sync.dma_start(slab[:, 0:S // 2],
                                  xvT[c * P:(c + 1) * P, 0:S // 2])
                nc.sync.dma_start(slab[:, S // 2:S],
                                  xvT[c * P:(c + 1) * P, S // 2:S])
                vsl.append(slab)
            for rc in range(NRC):
                ps = gp.tile([P, QW], F32, tag="gp")
                for c in range(NC):
                    nc.tensor.matmul(
                        ps[:, 0:VCOL],
                        vsl[c][:, rc * P:(rc + 1) * P],
                        wv_sb[:, c, :],
                        start=(c == 0), stop=False,
                    )
                nc.tensor.matmul(
                    ps[:, 0:VCOL],
                    ones1[:, :],
                    bvz_sb[:, :],
                    start=False, stop=True, skip_group_check=True,
                )
                if rc % 2 == 0:
                    nc.vector.tensor_copy(out=v4[:, rc, :], in_=ps[:, 0:VCOL])
                else:
                    nc.scalar.copy(out=v4[:, rc, :], in_=ps[:, 0:VCOL])

        # -------- phase 2: attention + output projection, per window --------
        with tc.tile_pool(name="st_ps", bufs=2, space="PSUM") as st_ps, \
             tc.tile_pool(name="ot_ps", bufs=2, space="PSUM") as ot_ps, \
             tc.tile_pool(name="op_ps", bufs=2, space="PSUM") as op_ps, \
             tc.tile_pool(name="smp", bufs=4) as smp, \
             tc.tile_pool(name="osb", bufs=4) as osb:
            for qm in range(NW):
                npair = 2 * qm + 2
                # ST + exp for all key-block pairs of this window
                for h in range(HG):
                    hp, hh = h // 2, h % 2
                    lo, hi = hh * DH, (hh + 1) * DH
                    for j in range(npair):
                        st = st_ps.tile([P, 2, QW], F32, tag="st")
                        v0p = None
                        for s in range(2):
                            kb = 2 * j + s
                            joff = kb - 4 * qm
                            v0 = max(joff, 0) * P
                            if s == 0:
                                v0p = v0
                            nc.tensor.matmul(
                                st[:, s, v0:QW],
                                kT[hp][lo:hi, kb * P:(kb + 1) * P],
                                qT[hp][lo:hi, qm * QW + v0:(qm + 1) * QW],
                                start=True, stop=(joff < 0),
                            )
                            if joff >= 0:
                                nc.tensor.matmul(
                                    st[:, s, v0:v0 + P],
                                    ident_sb[:],
                                    maskb_sb[:],
                                    start=False, stop=True,
                                    skip_group_check=True,
                                )
                        # one exp over both sub-blocks (flat view); leading
                        # fully-masked columns of the pair are skipped.
                        stf = st.rearrange("p s q -> p (s q)")
                        ptf = pt[h][j].rearrange("p s q -> p (s q)")
                        nc.scalar.activation(
                            ptf[:, v0p:2 * QW], stf[:, v0p:2 * QW],
                            mybir.ActivationFunctionType.Exp, scale=0.125)
                # AV + normalize + transpose per 128-query chunk
                for qb in range(4):
                    gq = 4 * qm + qb          # global query block
                    ot = ot_ps.tile([P, VCOL], F32, tag="av")
                    qlo = qb * P
                    for h in range(HG):
                        first = True
                        for kb in range(gq + 1):
                            j, s = kb // 2, kb % 2
                            nc.tensor.matmul(
                                ot[:, h * (DH + 1):(h + 1) * (DH + 1)],
                                pt[h][j][:, s, qlo:qlo + P],
                                v4[:, kb, h * (DH + 1):(h + 1) * (DH + 1)],
                                start=first, stop=(kb == gq),
                            )
                            first = False
                    # normalize: ao[q, h*64+d] = ot[q, h*65+d] / ot[q, h*65+64]
                    ot4 = ot[:, 0:VCOL].rearrange("p (h j) -> p h j", j=DH + 1)
                    rcp = smp.tile([P, HG], F32, tag="rcp")
                    nc.vector.reciprocal(rcp[:], ot4[:, :, DH])
                    ao = smp.tile([P, HG, DH], BF16, tag="ao")
                    nc.vector.tensor_tensor(
                        out=ao[:],
                        in0=ot4[:, :, 0:DH],
                        in1=rcp[:, :, None].broadcast_to([P, HG, DH]),
                        op=mybir.AluOpType.mult,
                    )
                    # transpose [128q, 256hd] -> aoT[:, :, gq cols]
                    aof = ao.rearrange("p h d -> p (h d)")
                    tr = ot_ps.tile([P, 2, P], BF16, tag="av")
                    for hp in range(2):
                        nc.tensor.transpose(tr[:, hp, :],
                                            aof[:, hp * P:(hp + 1) * P],
                                            ident_sb[:])
                    nc.vector.tensor_copy(
                        out=aoT[:, :, gq * P:(gq + 1) * P], in_=tr[:])
                # output projection for this window's 4 rowchunks
                for rc in range(4 * qm, 4 * qm + 4):
                    o_sb = osb.tile([P, D], BF16, tag="o_sb")
                    for nn in range(2):
                        pso = op_ps.tile([P, QW], F32, tag="pso")
                        for hp in range(2):
                            nc.tensor.matmul(
                                pso[:],
                                aoT[:, hp, rc * P:(rc + 1) * P],
                                wo_sb[:, hp, nn * QW:(nn + 1) * QW],
                                start=(hp == 0), stop=(hp == 1),
                            )
                        nc.vector.tensor_tensor(
                            out=o_sb[:, nn * QW:(nn + 1) * QW],
                            in0=pso[:],
                            in1=bo_bc[:, nn * QW:(nn + 1) * QW],
                            op=mybir.AluOpType.add,
                        )
                    nc.sync.dma_start(out[rc * P:(rc + 1) * P, :], o_sb[:])

    nc.compile()
    return nc


def _get_nc():
    global _cached_nc
    if _cached_nc is None:
        _cached_nc = _build_nc()
    return _cached_nc


def _shard_inputs(xk, xq, xv, wq, bq, wk, bk, wv, bv, wo, bo):
    f32 = np.float32
    maskb = np.zeros((P, P), f32)
    for k in range(P):
        maskb[k, :k] = -1.0e9
    maskb = maskb.astype(_bf16)
    identb = np.eye(P, dtype=f32).astype(_bf16)
    xqn = np.asarray(xq, f32)
    xkn = np.asarray(xk, f32)
    xvn = np.asarray(xv, f32)
    xqTb = [np.ascontiguousarray(xqn[b].T.astype(_bf16)) for b in range(B)]
    xkTb = [np.ascontiguousarray(xkn[b].T.astype(_bf16)) for b in range(B)]
    xvTb = [np.ascontiguousarray(xvn[b].T.astype(_bf16)) for b in range(B)]
    wqn = np.asarray(wq, f32)
    wkn = np.asarray(wk, f32)
    wvn = np.asarray(wv, f32)
    won = np.asarray(wo, f32)
    bqn = np.asarray(bq, f32)
    bkn = np.asarray(bk, f32)
    bvn = np.asarray(bv, f32)
    bon = np.asarray(bo, f32)
    in_maps = []
    for c in range(8):
        b, g = divmod(c, 4)
        gs = slice(g * DG, (g + 1) * DG)
        # wv extended: per head 64 v-cols + 1 zero col; bvz = bias + ones col
        wve = np.zeros((D, HG * (DH + 1)), f32)
        bvz = np.zeros((1, HG * (DH + 1)), f32)
        for h in range(HG):
            cs = g * DG + h * DH
            wve[:, h * (DH + 1):h * (DH + 1) + DH] = wvn[:, cs:cs + DH]
            bvz[0, h * (DH + 1):h * (DH + 1) + DH] = bvn[cs:cs + DH]
            bvz[0, h * (DH + 1) + DH] = 1.0
        in_maps.append({
            "xqT": xqTb[b],
            "xkT": xkTb[b],
            "xvT": xvTb[b],
            "wq": np.ascontiguousarray(wqn[:, gs].astype(_bf16)),
            "wk": np.ascontiguousarray(wkn[:, gs].astype(_bf16)),
            "wve": np.ascontiguousarray(wve.astype(_bf16)),
            "bq": np.ascontiguousarray(bqn[gs]),
            "bk": np.ascontiguousarray(bkn[gs]),
            "bvz": bvz,
            "wo": np.ascontiguousarray(won[gs, :].astype(_bf16)),
            "bo": bon if g == 0 else np.zeros(D, f32),
            "maskb": maskb,
            "identb": identb,
            "onesd": np.ones((1, P), f32),
        })
    return in_maps


def kernel(xk, xq, xv, wq, bq, wk, bk, wv, bv, wo, bo, _trace=False):
    nc = _get_nc()
    in_maps = _shard_inputs(xk, xq, xv, wq, bq, wk, bk, wv, bv, wo, bo)
    res = run_bass_kernel_spmd(nc, in_maps, core_ids=list(range(8)),
                               trace=_trace)
    parts = [np.asarray(r["out"], np.float32) for r in res.results]
    out = np.stack([
        parts[0] + parts[1] + parts[2] + parts[3],
        parts[4] + parts[5] + parts[6] + parts[7],
    ]).astype(np.float32)
    if _trace:
        kernel._last_results = res
    return out


# revision 11
# speedup vs baseline: 1.1803x; 1.1803x over previous
"""Causal-mask multi-head attention (B=2, S=2048, D=1024, H=16) on 8 trn2
NeuronCores.

Sharding: core c = 4*b + g handles batch b and head-group g (4 heads).
Host sums the 4 per-batch partials and adds bo -- no device collectives.

Kernel structure (v3):
  - Software-pipelined per 512-query window: ST(w) -> proj(w+1) -> AV(w) ->
    transpose(w) -> out-proj(w), so projection matmuls fill the PE bubble
    while exp(w) runs on ACT/Pool/DVE.
  - All matmuls bf16.  ST per key-block PAIR into one [128, 2, 512] psum
    tile; one exp covers 1024 columns.
  - Causal mask on diagonal blocks via bf16 ident@maskb PE accumulation;
    exp of masked lanes underflows to 0.
  - exp split across engines: diagonal pairs use the ACT exp; off-diagonal
    pairs use a bf16 bit-trick fast-exp (y=round(x*a+b) as int16 == bf16 of
    2^y) on Pool/DVE/ACT in rotation (~3% rel err, cancels in softmax).
  - AV "orientation B": stationary P^T chunk [128k,128q], moving v4
    [128k,65] -> psum [128q, 64+Z]; per-partition normalize, PE transpose
    back to [hd, q] for the output projection.
"""
import numpy as np
import ml_dtypes

_bf16 = ml_dtypes.bfloat16

import concourse.bass as bass
import concourse.tile as tile
import concourse.mybir as mybir
from concourse import bacc
from concourse.bass_utils import run_bass_kernel_spmd

B, S, D = 2, 2048, 1024
H, DH = 16, 64
HG = 4                 # heads per core
DG = HG * DH           # 256 projection cols per core
P = 128
QW = 512               # query window
NW = S // QW           # 4 windows
NKB = S // P           # 16 key blocks
NC = D // P            # 8 contraction chunks of d_model
NRC = S // P           # 16 row chunks
F32 = mybir.dt.float32
F32R = mybir.dt.float32r
BF16 = mybir.dt.bfloat16
I16 = mybir.dt.int16
VCOL = HG * (DH + 1)   # 260

LOG2E = float(np.log2(np.e))
FEXP_A = LOG2E * 128.0 * 0.125      # folds the 1/sqrt(dh) logit scale
FEXP_B = 127.0 * 128.0 - 6.0

_cached_nc = None


def _build_nc():
    nc = bacc.Bacc("TRN2", target_bir_lowering=False, debug=False, num_devices=8)

    xqT = nc.dram_tensor("xqT", [D, S], BF16, kind="ExternalInput").ap()
    xkT = nc.dram_tensor("xkT", [D, S], BF16, kind="ExternalInput").ap()
    xvT = nc.dram_tensor("xvT", [D, S], BF16, kind="ExternalInput").ap()
    wq = nc.dram_tensor("wq", [D, DG], BF16, kind="ExternalInput").ap()
    wk = nc.dram_tensor("wk", [D, DG], BF16, kind="ExternalInput").ap()
    wve = nc.dram_tensor("wve", [D, VCOL], BF16, kind="ExternalInput").ap()
    bq = nc.dram_tensor("bq", [DG], F32, kind="ExternalInput").ap()
    bk = nc.dram_tensor("bk", [DG], F32, kind="ExternalInput").ap()
    bvz = nc.dram_tensor("bvz", [1, VCOL], F32R, kind="ExternalInput").ap()
    wo = nc.dram_tensor("wo", [DG, D], BF16, kind="ExternalInput").ap()
    maskb = nc.dram_tensor("maskb", [P, P], BF16, kind="ExternalInput").ap()
    identb = nc.dram_tensor("identb", [P, P], BF16, kind="ExternalInput").ap()
    onesd = nc.dram_tensor("onesd", [1, P], F32R, kind="ExternalInput").ap()
    out = nc.dram_tensor("out", [S, D], BF16, kind="ExternalOutput").ap()

    from contextlib import ExitStack
    with tile.TileContext(nc) as tc, ExitStack() as ctx:
        consts = ctx.enter_context(tc.tile_pool(name="consts", bufs=1))
        slabs = ctx.enter_context(tc.tile_pool(name="slabs", bufs=1))
        persist = ctx.enter_context(tc.tile_pool(name="persist", bufs=1))
        ptp = ctx.enter_context(tc.tile_pool(name="ptp", bufs=1))

        # ---- weights + first-half slabs (issue order = need order) ----
        wk_sb = consts.tile([P, NC, DG], BF16)
        nc.sync.dma_start(wk_sb[:], wk.rearrange("(c p) m -> p c m", p=P))
        # shared 32-slot slab ring: first-half k/q/v slabs live until their
        # projections finish, then second-half slabs recycle the slots.
        def mksl(nm):
            return slabs.tile([P, S // 2], BF16, tag="sl", bufs=32, name=nm)
        ksl = [mksl(f"ksl{c}") for c in range(NC)]
        qsl = [mksl(f"qsl{c}") for c in range(NC)]
        vsl = [mksl(f"vsl{c}") for c in range(NC)]
        ksl2 = [mksl(f"ksl2{c}") for c in range(NC)]
        qsl2 = [mksl(f"qsl2{c}") for c in range(NC)]
        vsl2 = [mksl(f"vsl2{c}") for c in range(NC)]
        for c in range(NC):
            nc.sync.dma_start(ksl[c][:], xkT[c * P:(c + 1) * P, 0:S // 2])
        wq_sb = consts.tile([P, NC, DG], BF16)
        nc.sync.dma_start(wq_sb[:], wq.rearrange("(c p) m -> p c m", p=P))
        for c in range(NC):
            nc.sync.dma_start(qsl[c][:], xqT[c * P:(c + 1) * P, 0:S // 2])
        maskb_sb = consts.tile([P, P], BF16)
        nc.sync.dma_start(maskb_sb[:], maskb)
        ident_sb = consts.tile([P, P], BF16)
        nc.sync.dma_start(ident_sb[:], identb)
        bk_sb = consts.tile([P, 2], F32)
        nc.sync.dma_start(bk_sb[:], bk.rearrange("(t p) -> p t", p=P))
        bq_sb = consts.tile([P, 2], F32)
        nc.sync.dma_start(bq_sb[:], bq.rearrange("(t p) -> p t", p=P))
        wv_sb = consts.tile([P, NC, VCOL], BF16)
        nc.sync.dma_start(wv_sb[:], wve.rearrange("(c p) m -> p c m", p=P))
        for c in range(NC):
            nc.sync.dma_start(vsl[c][:], xvT[c * P:(c + 1) * P, 0:S // 2])
        bvz_sb = consts.tile([1, VCOL], F32R)
        nc.sync.dma_start(bvz_sb[:], bvz)
        ones1 = consts.tile([1, P], F32R)
        nc.sync.dma_start(ones1[:], onesd)
        wo_sb = consts.tile([P, 2, D], BF16)
        nc.sync.dma_start(wo_sb[:], wo.rearrange("(t p) m -> p t m", p=P))
        for c in range(NC):
            nc.sync.dma_start(ksl2[c][:], xkT[c * P:(c + 1) * P, S // 2:S])
        for c in range(NC):
            nc.sync.dma_start(qsl2[c][:], xqT[c * P:(c + 1) * P, S // 2:S])
        for c in range(NC):
            nc.sync.dma_start(vsl2[c][:], xvT[c * P:(c + 1) * P, S // 2:S])

        def kslab(c, w):
            sl = (ksl, ksl2)[w // 2][c]
            return sl[:, (w % 2) * QW:(w % 2) * QW + QW]

        def qslab(c, w):
            sl = (qsl, qsl2)[w // 2][c]
            return sl[:, (w % 2) * QW:(w % 2) * QW + QW]

        def vslab(c, rc):
            sl = (vsl, vsl2)[rc // 8][c]
            return sl[:, (rc % 8) * P:(rc % 8) * P + P]

        # persistent activation storage
        qT = [persist.tile([P, S], BF16, tag=f"qT{t}", name=f"qT{t}") for t in range(2)]
        kT = [persist.tile([P, S], BF16, tag=f"kT{t}", name=f"kT{t}") for t in range(2)]
        v4 = persist.tile([P, NRC, VCOL], BF16, tag="v4")
        aoT = persist.tile([P, 2, S], BF16, tag="aoT")
        pt = [[ptp.tile([P, 2, QW], BF16, tag=f"pt{h}_{j}", name=f"pt{h}_{j}")
               for j in range(NKB // 2)] for h in range(HG)]

        with tc.tile_pool(name="gp", bufs=2, space="PSUM") as gp, \
             tc.tile_pool(name="st_ps", bufs=2, space="PSUM") as st_ps, \
             tc.tile_pool(name="av_ps", bufs=2, space="PSUM") as av_ps, \
             tc.tile_pool(name="smp", bufs=4) as smp, \
             tc.tile_pool(name="osb", bufs=4) as osb:

            def proj_window(w):
                # k then q: psum rows = head dims (2 tiles of 128), cols = w
                for which, w_sb, b_sb, dst, sfn in (
                    ("k", wk_sb, bk_sb, kT, kslab),
                    ("q", wq_sb, bq_sb, qT, qslab),
                ):
                    for t in range(2):
                        ps = gp.tile([P, QW], F32, tag="gp", name=f"ps_{which}{t}")
                        for c in range(NC):
                            nc.tensor.matmul(
                                ps[:],
                                w_sb[:, c, t * P:(t + 1) * P],
                                sfn(c, w),
                                start=(c == 0), stop=(c == NC - 1),
                            )
                        nc.vector.tensor_scalar_add(
                            dst[t][:, w * QW:(w + 1) * QW],
                            ps[:], b_sb[:, t:t + 1])
                # v rowchunks of this window
                for rc in range(4 * w, 4 * w + 4):
                    ps = gp.tile([P, QW], F32, tag="gp", name="ps_v")
                    for c in range(NC):
                        nc.tensor.matmul(
                            ps[:, 0:VCOL],
                            vslab(c, rc),
                            wv_sb[:, c, :],
                            start=(c == 0), stop=False,
                        )
                    nc.tensor.matmul(
                        ps[:, 0:VCOL],
                        ones1[:, :],
                        bvz_sb[:, :],
                        start=False, stop=True, skip_group_check=True,
                    )
                    nc.vector.tensor_copy(out=v4[:, rc, :], in_=ps[:, 0:VCOL])

            def st_exp_window(qm):
                npair = 2 * qm + 2
                noff = 0
                for h in range(HG):
                    hp, hh = h // 2, h % 2
                    lo, hi = hh * DH, (hh + 1) * DH
                    for j in range(npair):
                        st = st_ps.tile([P, 2, QW], F32, tag="st")
                        diag = (2 * j + 1 - 4 * qm) >= 0
                        v0p = None
                        for s in range(2):
                            kb = 2 * j + s
                            joff = kb - 4 * qm
                            v0 = max(joff, 0) * P
                            if s == 0:
                                v0p = v0
                            nc.tensor.matmul(
                                st[:, s, v0:QW],
                                kT[hp][lo:hi, kb * P:(kb + 1) * P],
                                qT[hp][lo:hi, qm * QW + v0:(qm + 1) * QW],
                                start=True, stop=(joff < 0),
                            )
                            if joff >= 0:
                                nc.tensor.matmul(
                                    st[:, s, v0:v0 + P],
                                    ident_sb[:],
                                    maskb_sb[:],
                                    start=False, stop=True,
                                    skip_group_check=True,
                                )
                        stf = st.rearrange("p s q -> p (s q)")
                        ptf = pt[h][j].rearrange("p s q -> p (s q)")
                        if diag:
                            # true exp on ACT (handles -1e9 -> 0)
                            nc.scalar.activation(
                                ptf[:, v0p:2 * QW], stf[:, v0p:2 * QW],
                                mybir.ActivationFunctionType.Exp, scale=0.125)
                        elif noff % 5 < 2:
                            # fast-exp bit trick on DVE (gpsimd can't read
                            # PSUM); ~3.5% rel err, cancels in softmax
                            noff += 1
                            nc.vector.tensor_scalar(
                                out=ptf.bitcast(I16),
                                in0=stf[:],
                                scalar1=FEXP_A, scalar2=FEXP_B,
                                op0=mybir.AluOpType.mult,
                                op1=mybir.AluOpType.add)
                        else:
                            noff += 1
                            nc.scalar.activation(
                                ptf[:], stf[:],
                                mybir.ActivationFunctionType.Exp, scale=0.125)

            def av_window(qm):
                for qb in range(4):
                    gq = 4 * qm + qb
                    ot = av_ps.tile([P, VCOL], F32, tag="av", name="ot")
                    qlo = qb * P
                    for h in range(HG):
                        for kb in range(gq + 1):
                            j, s = kb // 2, kb % 2
                            nc.tensor.matmul(
                                ot[:, h * (DH + 1):(h + 1) * (DH + 1)],
                                pt[h][j][:, s, qlo:qlo + P],
                                v4[:, kb, h * (DH + 1):(h + 1) * (DH + 1)],
                                start=(kb == 0), stop=(kb == gq),
                            )
                    ot4 = ot[:, 0:VCOL].rearrange("p (h j) -> p h j", j=DH + 1)
                    rcp = smp.tile([P, HG], F32, tag="rcp")
                    nc.vector.reciprocal(rcp[:], ot4[:, :, DH])
                    ao = smp.tile([P, HG, DH], BF16, tag="ao")
                    nc.vector.tensor_tensor(
                        out=ao[:],
                        in0=ot4[:, :, 0:DH],
                        in1=rcp[:, :, None].broadcast_to([P, HG, DH]),
                        op=mybir.AluOpType.mult,
                    )
                    aof = ao.rearrange("p h d -> p (h d)")
                    tr = av_ps.tile([P, 2, P], BF16, tag="av", name="tr")
                    for hp in range(2):
                        nc.tensor.transpose(tr[:, hp, :],
                                            aof[:, hp * P:(hp + 1) * P],
                                            ident_sb[:])
                    nc.scalar.copy(
                        out=aoT[:, :, gq * P:(gq + 1) * P], in_=tr[:])

            def outproj_window(qm):
                for i, rc in enumerate(range(4 * qm, 4 * qm + 4)):
                    o_sb = osb.tile([P, D], BF16, tag="o_sb")
                    for nn in range(2):
                        pso = gp.tile([P, QW], F32, tag="gp", name="pso")
                        for hp in range(2):
                            nc.tensor.matmul(
                                pso[:],
                                aoT[:, hp, rc * P:(rc + 1) * P],
                                wo_sb[:, hp, nn * QW:(nn + 1) * QW],
                                start=(hp == 0), stop=(hp == 1),
                            )
                        if (2 * i + nn) % 2 == 0:
                            nc.scalar.copy(out=o_sb[:, nn * QW:(nn + 1) * QW],
                                           in_=pso[:])
                        else:
                            nc.vector.tensor_copy(
                                out=o_sb[:, nn * QW:(nn + 1) * QW],
                                in_=pso[:])
                    nc.sync.dma_start(out[rc * P:(rc + 1) * P, :], o_sb[:])

            proj_window(0)
            for w in range(NW):
                st_exp_window(w)
                if w + 1 < NW:
                    proj_window(w + 1)
                av_window(w)
                outproj_window(w)

    nc.compile()
    return nc


def _get_nc():
    global _cached_nc
    if _cached_nc is None:
        _cached_nc = _build_nc()
    return _cached_nc


def _shard_inputs(xk, xq, xv, wq, bq, wk, bk, wv, bv, wo, bo):
    f32 = np.float32
    maskb = np.zeros((P, P), f32)
    for k in range(P):
        maskb[k, :k] = -1.0e9
    maskb = maskb.astype(_bf16)
    identb = np.eye(P, dtype=f32).astype(_bf16)
    xqn = np.asarray(xq, f32)
    xkn = np.asarray(xk, f32)
    xvn = np.asarray(xv, f32)
    xqTb = [np.ascontiguousarray(xqn[b].T.astype(_bf16)) for b in range(B)]
    xkTb = [np.ascontiguousarray(xkn[b].T.astype(_bf16)) for b in range(B)]
    xvTb = [np.ascontiguousarray(xvn[b].T.astype(_bf16)) for b in range(B)]
    wqn = np.asarray(wq, f32)
    wkn = np.asarray(wk, f32)
    wvn = np.asarray(wv, f32)
    won = np.asarray(wo, f32)
    bqn = np.asarray(bq, f32)
    bkn = np.asarray(bk, f32)
    bvn = np.asarray(bv, f32)
    in_maps = []
    for c in range(8):
        b, g = divmod(c, 4)
        gs = slice(g * DG, (g + 1) * DG)
        wve = np.zeros((D, VCOL), f32)
        bvz = np.zeros((1, VCOL), f32)
        for h in range(HG):
            cs = g * DG + h * DH
            wve[:, h * (DH + 1):h * (DH + 1) + DH] = wvn[:, cs:cs + DH]
            bvz[0, h * (DH + 1):h * (DH + 1) + DH] = bvn[cs:cs + DH]
            bvz[0, h * (DH + 1) + DH] = 1.0
        in_maps.append({
            "xqT": xqTb[b],
            "xkT": xkTb[b],
            "xvT": xvTb[b],
            "wq": np.ascontiguousarray(wqn[:, gs].astype(_bf16)),
            "wk": np.ascontiguousarray(wkn[:, gs].astype(_bf16)),
            "wve": np.ascontiguousarray(wve.astype(_bf16)),
            "bq": np.ascontiguousarray(bqn[gs]),
            "bk": np.ascontiguousarray(bkn[gs]),
            "bvz": bvz,
            "wo": np.ascontiguousarray(won[gs, :].astype(_bf16)),
            "maskb": maskb,
            "identb": identb,
            "onesd": np.ones((1, P), f32),
        })
    return in_maps


def kernel(xk, xq, xv, wq, bq, wk, bk, wv, bv, wo, bo, _trace=False):
    nc = _get_nc()
    in_maps = _shard_inputs(xk, xq, xv, wq, bq, wk, bk, wv, bv, wo, bo)
    res = run_bass_kernel_spmd(nc, in_maps, core_ids=list(range(8)),
                               trace=_trace)
    parts = [np.asarray(r["out"], np.float32) for r in res.results]
    bon = np.asarray(bo, np.float32)
    out = np.stack([
        parts[0] + parts[1] + parts[2] + parts[3] + bon,
        parts[4] + parts[5] + parts[6] + parts[7] + bon,
    ]).astype(np.float32)
    if _trace:
        kernel._last_results = res
    return out
